# revision 45
# baseline (speedup 1.0000x reference)
"""DiffusionTransformerLayer on 8 Trainium2 NeuronCores.

Sharding: rows (B*N = 2048 tokens) split 256/core; attention K/V all-gathered
within each batch's 4-core group (one 4-rank AllGather of fp8 k^T/v).

Layout strategy:
  - ALL host inputs packed into ONE flat bf16-typed DRAM blob per core
    (fp32/fp8 regions stored as raw bytes, bitcast on device): per-iteration
    dispatch cost is dominated by argument count, not device time.
  - weights are fp8(e4m3), host-prescaled by S8=32 (consumers fold 1/S8 back
    in via activation scale= or scalar_tensor_tensor). Transposed activations
    (sT/snT/bT/a2T, hT, xT) are also fp8, enabling DoubleRow matmuls (two
    128-deep K-subtiles per instruction, 2x PE throughput) for every large
    projection: adaln/gates, q/k/v/g, o_w, transition, b2a.
  - "natural" activations: [rows(part), feat(free)]; LN / gates / residuals
    bf16/fp32; residual stream fp32; PSUM always fp32.
  - attention runs feature-on-partition: heads padded 48->64 at partition
    bases 0/64. Scores transposed (ST[k,q]); softmax over k = exp on ACT
    (scale folds HD^-0.5/S8^2), pair-bias via elementwise multiply with
    host-precomputed exp(z^T) in fp8 (split DVE/Pool), row-sums arrive as a
    (S8/256)-scaled ones-column through the padded-V matmul so the p-sum rows
    stay fp8-safe in xT.
  - DMA is batched: each weight arrives in ONE multi-dim-strided descriptor
    (~57 DMAs total vs ~200 naive; HWDGE is descriptor-count bound).
  - elementwise work is split across DVE and Pool (gpsimd); Pool never
    touches PSUM (hardware restriction).
"""
import os

import numpy as np
import ml_dtypes

import concourse.bacc as bacc
import concourse.bass as bass
import concourse.tile as tile
from concourse import mybir
from concourse.bass_utils import run_bass_kernel_spmd

F32 = mybir.dt.float32
BF16 = mybir.dt.bfloat16
FP8 = mybir.dt.float8e4
AF = mybir.ActivationFunctionType
OP = mybir.AluOpType
NPBF16 = ml_dtypes.bfloat16
NPF8 = ml_dtypes.float8_e4m3

B, N, D, H = 2, 1024, 768, 16
HD, HDP = 48, 64
HP = H * HDP          # 1024 padded head dims
HID = 1536
R = 256               # rows per core
FC = D // 128         # 6 feature chunks
EPS = 1e-5
SCALE = HD ** -0.5
S8 = 32.0             # fp8 weight pre-scale (host); consumers divide back out
IS = 1.0 / S8
IS2 = 1.0 / (S8 ** 2)
PM = mybir.MatmulPerfMode
KT_ELEMS = HP * R     # kT shard elems (fp8: half as many bf16 slots)
V_ELEMS = R * HP      # v shard elems (fp8)
KT_SLOTS = KT_ELEMS // 2
SHARD = KT_SLOTS + V_ELEMS // 2

_NC_CACHE = {}

# Single packed input blob (bf16-typed; fp32/fp8 regions stored as raw bytes
# and bitcast on device). (name, shape, kind). Order = blob layout.
_LAYOUT = [
    ("a", (R, D), "f32"),
    ("qb", (8, 128), "f32"),
    ("s", (R, D), "bf16"),
    ("ez", (H, 128, 8, R), "fp8"),
    ("ident", (128, 128), "bf16"),
    ("sel", (4, 4, 256), "bf16"),
    ("vones", (1, HP), "bf16"),
    ("w_ss1", (D + 1, D), "fp8"),
    ("w_sb1", (D, D), "fp8"),
    ("w_ss2", (D + 1, D), "fp8"),
    ("w_sb2", (D, D), "fp8"),
    ("w_q", (D, HP), "fp8"),
    ("w_k", (D, HP), "fp8"),
    ("w_g", (D, HP), "fp8"),
    ("w_v", (D, HP), "fp8"),
    ("w_ow", (HP, D), "fp8"),
    ("w_op1", (D + 1, D), "fp8"),
    ("w_op2", (D + 1, D), "fp8"),
    ("w_swu", (D, HID), "fp8"),
    ("w_swg", (D, HID), "fp8"),
    ("w_ab", (D, HID), "fp8"),
    ("w_ba", (HID, D), "fp8"),
]


def _layout_offsets():
    offs, off = {}, 0
    for name, shape, kind in _LAYOUT:
        n = int(np.prod(shape))
        slots = {"f32": 2 * n, "bf16": n, "fp8": n // 2}[kind]
        assert kind != "fp8" or n % 2 == 0
        offs[name] = (off, slots, shape, kind)
        off += slots
    return offs, off


_OFFS, _BLOB_LEN = _layout_offsets()


def _build_nc():
    stage = int(os.environ.get("KSTAGE", "4"))
    nc = bacc.Bacc("TRN2", target_bir_lowering=False, debug=False, num_devices=8)

    blob = nc.dram_tensor("blob", [_BLOB_LEN], BF16, kind="ExternalInput")

    def vin(name):
        off, slots, shape, kind = _OFFS[name]
        ap = blob.ap()[off:off + slots]
        if kind == "f32":
            ap = ap.bitcast(F32)
        elif kind == "fp8":
            ap = ap.bitcast(FP8)
        dims = " ".join(f"d{i}" for i in range(len(shape)))
        kw = {f"d{i}": shape[i] for i in range(1, len(shape))}
        return ap.rearrange(f"({dims}) -> {dims}", **kw)

    a_in = vin("a")
    s_in = vin("s")
    ez_in = vin("ez")
    ident_in = vin("ident")
    sel_in = vin("sel")
    vones_in = vin("vones")
    qb_in = vin("qb")
    w_ss1 = vin("w_ss1")
    w_sb1 = vin("w_sb1")
    w_ss2 = vin("w_ss2")
    w_sb2 = vin("w_sb2")
    w_q = vin("w_q")
    w_k = vin("w_k")
    w_g = vin("w_g")
    w_v = vin("w_v")
    w_ow = vin("w_ow")
    w_op1 = vin("w_op1")
    w_op2 = vin("w_op2")
    w_swu = vin("w_swu")
    w_swg = vin("w_swg")
    w_ab = vin("w_ab")
    w_ba = vin("w_ba")

    y_out = nc.dram_tensor("y", [R, D], F32, kind="ExternalOutput")

    with tile.TileContext(nc) as tc:
        from contextlib import ExitStack
        with ExitStack() as ctx:
            cst = ctx.enter_context(tc.tile_pool(name="cst", bufs=1))
            acts = ctx.enter_context(tc.tile_pool(name="acts", bufs=1))
            lnp = ctx.enter_context(tc.tile_pool(name="lnp", bufs=3))
            wp = ctx.enter_context(tc.tile_pool(name="wp", bufs=4))
            wcb = ctx.enter_context(tc.tile_pool(name="wcb", bufs=1))
            ezp = ctx.enter_context(tc.tile_pool(name="ezp", bufs=4))
            pp = ctx.enter_context(tc.tile_pool(name="pp", bufs=6))
            tmp = ctx.enter_context(tc.tile_pool(name="tmp", bufs=2))
            dram = ctx.enter_context(tc.tile_pool(name="dram", bufs=1, space="DRAM"))
            ps2 = ctx.enter_context(tc.tile_pool(name="ps2", bufs=2, space="PSUM"))
            ps1 = ctx.enter_context(tc.tile_pool(name="ps1", bufs=4, space="PSUM"))

            # ---------------- load s (+ident) first: feeds the first PE work --
            a_sb, s_sb = [], []
            for rt in range(2):
                st = acts.tile([128, D], BF16, tag=f"s{rt}")
                nc.sync.dma_start(out=st, in_=s_in[128 * rt:128 * (rt + 1), :])
                s_sb.append(st)
            ident = cst.tile([128, 128], BF16)
            nc.sync.dma_start(out=ident, in_=ident_in)
            for rt in range(2):
                at = acts.tile([128, D], F32, tag=f"a{rt}")
                nc.sync.dma_start(out=at, in_=a_in[128 * rt:128 * (rt + 1), :])
                a_sb.append(at)

            # ---------------- constants ----------------
            vones = cst.tile([1, HP], BF16)
            nc.sync.dma_start(out=vones, in_=vones_in)
            qb = cst.tile([128, 8], F32)
            nc.sync.dma_start(out=qb, in_=qb_in.rearrange("t p -> p t"))
            eps_t = cst.tile([128, 1], F32)
            nc.vector.memset(eps_t, EPS)
            ones1 = cst.tile([1, 256], BF16)
            nc.vector.memset(ones1, 1.0)
            onesf = cst.tile([128, 64], BF16)
            nc.vector.memset(onesf, 1.0)

            # ---------------- helpers ----------------
            def layernorm(dst, src, rts=(0, 1)):
                """dst[rt] = LN(src[rt]) without affine, bf16 out (DVE)."""
                for rt in rts:
                    stats = lnp.tile([128, 3, 6], F32, tag="lnstats")
                    mv = lnp.tile([128, 2], F32, tag="lnmv")
                    src3 = src[rt].rearrange("p (s c) -> p s c", s=3)
                    for sg in range(3):
                        nc.vector.bn_stats(out=stats[:, sg, :], in_=src3[:, sg, :])
                    nc.vector.bn_aggr(out=mv, in_=stats)
                    rstd = lnp.tile([128, 1], F32, tag="lnrstd")
                    nc.scalar.activation(out=rstd, in_=mv[:, 1:2], func=AF.Sqrt,
                                         bias=eps_t, scale=1.0)
                    nc.vector.reciprocal(out=rstd, in_=rstd)
                    nc.vector.tensor_scalar(out=dst[rt], in0=src[rt],
                                            scalar1=mv[:, 0:1], scalar2=rstd,
                                            op0=OP.subtract, op1=OP.mult)

            def layernorm_pool(dst, src, rts=(0, 1)):
                """LN on the Pool engine (no bn_stats there): mean via reduce,
                centered square with accum_out for the variance; the tiny
                [128,1] rstd ops ride on ACT/DVE."""
                for rt in rts:
                    mscr = lnp.tile([128, D], BF16, tag="plnms", bufs=2)
                    mean = lnp.tile([128, 1], F32, tag="plnmean")
                    nc.gpsimd.tensor_scalar(out=mscr, in0=src[rt],
                                            scalar1=1.0 / D, scalar2=None,
                                            op0=OP.mult, accum_out=mean)
                    ctr = lnp.tile([128, D], F32, tag="plnctr", bufs=2)
                    nc.gpsimd.tensor_scalar(out=ctr, in0=src[rt], scalar1=mean,
                                            scalar2=None, op0=OP.subtract)
                    sq = lnp.tile([128, D], BF16, tag="plnsq", bufs=2)
                    vsum = lnp.tile([128, 1], F32, tag="plnvs")
                    nc.gpsimd.scalar_tensor_tensor(
                        out=sq, in0=ctr, scalar=1.0, in1=ctr,
                        op0=OP.mult, op1=OP.mult, accum_out=vsum)
                    rstd = lnp.tile([128, 1], F32, tag="plnrstd")
                    nc.scalar.activation(out=rstd, in_=vsum, func=AF.Sqrt,
                                         bias=eps_t, scale=1.0 / D)
                    nc.vector.reciprocal(out=rstd, in_=rstd)
                    nc.gpsimd.tensor_scalar_mul(dst[rt], ctr, rstd)

            def transpose6(dst6, src, dve_every=2):
                """src: 2x[128,768] bf16 -> dst6: [128, FC, 256] fp8 (transposed)."""
                i = 0
                for rt in range(2):
                    for fc in range(FC):
                        pst = ps1.tile([128, 128], BF16, tag="ps1", name="pst")
                        nc.tensor.transpose(
                            out=pst, in_=src[rt][:, 128 * fc:128 * (fc + 1)],
                            identity=ident)
                        dslice = dst6[:, fc, 128 * rt:128 * (rt + 1)]
                        if i % dve_every == 0:
                            nc.vector.tensor_copy(out=dslice, in_=pst)
                        else:
                            nc.scalar.copy(out=dslice, in_=pst)
                        i += 1

            def proj_nat(lhsT, w_dram, n_fc, out_cols, bias_row=False, wtag="w768"):
                """Natural-orientation projection: returns 2 PSUM tiles [128,out_cols].

                lhsT: list of transposed-activation tiles [128, 256].
                Whole weight [n_fc*128, out_cols] arrives in ONE DMA as a
                [128, n_fc, out_cols] tile (chunk-major free dim).
                """
                pss = [ps2.tile([128, out_cols], F32, tag="ps2", name="ps_nat") for _ in range(2)]
                ncol = [(c, min(c + 512, out_cols)) for c in range(0, out_cols, 512)]
                wt = wp.tile([128, n_fc, out_cols], FP8, tag=wtag, bufs=2, name=f"wt_{wtag}")
                nc.sync.dma_start(
                    out=wt,
                    in_=w_dram[0:n_fc * 128, :].rearrange("(f p) c -> p f c", p=128))
                for fc in range(0, n_fc, 2):
                    for rt in range(2):
                        for (cs, ce) in ncol:
                            nc.tensor.matmul(
                                out=pss[rt][:, cs:ce],
                                lhsT=lhsT[:, fc:fc + 2, 128 * rt:128 * (rt + 1)],
                                rhs=wt[:, fc:fc + 2, cs:ce],
                                start=(fc == 0),
                                stop=(fc == n_fc - 2 and not bias_row),
                                perf_mode=PM.DoubleRow)
                if bias_row:
                    bt = wp.tile([1, out_cols], FP8, tag="wbias", name="wt_bias")
                    nc.sync.dma_start(out=bt, in_=w_dram[n_fc * 128:n_fc * 128 + 1, :])
                    for rt in range(2):
                        for (cs, ce) in ncol:
                            nc.tensor.matmul(
                                out=pss[rt][:, cs:ce],
                                lhsT=ones1[:, 128 * rt:128 * rt + 128],
                                rhs=bt[:, cs:ce],
                                start=False, stop=True)
                return pss

            # hoisted: s^T and both sigmoid output gates (independent of attention)
            sT6 = acts.tile([128, FC, 256], FP8, tag="sT6", name="sT6")
            transpose6(sT6, s_sb, dve_every=3)

            # ---------------- AdaLN 1 ----------------
            sn = [acts.tile([128, D], BF16, tag="lnout", bufs=4, name=f"sn{rt}") for rt in range(2)]
            an = [acts.tile([128, D], BF16, tag="lnout", bufs=4, name=f"an{rt}") for rt in range(2)]
            layernorm(sn, s_sb)
            layernorm(an, a_sb)

            snT6 = acts.tile([128, FC, 256], FP8, tag="snT6", name="snT6")
            transpose6(snT6, sn, dve_every=3)

            ps_ss1 = proj_nat(snT6, w_ss1, FC, D, bias_row=True)
            sig1 = [acts.tile([128, D], BF16, tag=f"sig_{rt}", bufs=1, name=f"sig1_{rt}") for rt in range(2)]
            for rt in range(2):
                nc.scalar.activation(out=sig1[rt], in_=ps_ss1[rt], func=AF.Sigmoid, scale=IS)

            ps_sb1 = proj_nat(snT6, w_sb1, FC, D)
            b_sb = [acts.tile([128, D], BF16, tag=f"ba2_{rt}", bufs=1, name=f"b{rt}") for rt in range(2)]
            for rt in range(2):
                tt = tmp.tile([128, D], BF16, tag="ttmp")
                nc.vector.tensor_mul(tt, an[rt], sig1[rt])
                nc.vector.scalar_tensor_tensor(out=b_sb[rt], in0=ps_sb1[rt], scalar=IS, in1=tt, op0=OP.mult, op1=OP.add)

            if stage == 1:
                for rt in range(2):
                    yt = tmp.tile([128, D], F32, tag="yt", bufs=2)
                    nc.vector.tensor_copy(out=yt, in_=b_sb[rt])
                    nc.sync.dma_start(out=y_out.ap()[128 * rt:128 * (rt + 1), :], in_=yt)
            if stage >= 2:
                bT6 = acts.tile([128, FC, 256], FP8, tag="bT6", name="bT6")
                transpose6(bT6, b_sb)

                # ---------------- k^T, v (pre-collective) ----------------
                wk_sb = wcb.tile([128, FC, HP], FP8, tag="wcb1024", bufs=3, name="wk")
                nc.sync.dma_start(out=wk_sb,
                                  in_=w_k.rearrange("(f p) c -> p f c", p=128))
                ktall = acts.tile([128, 8, 256], FP8, tag="ktall", name="ktall")
                for t in range(8):
                    ps = ps1.tile([128, 256], F32, tag="ps1", name="ps_cb")
                    for fc in range(0, FC, 2):
                        nc.tensor.matmul(out=ps,
                                         lhsT=wk_sb[:, fc:fc + 2, 128 * t:128 * (t + 1)],
                                         rhs=bT6[:, fc:fc + 2, :],
                                         start=(fc == 0), stop=(fc == FC - 2),
                                         perf_mode=PM.DoubleRow)
                    if t % 2 == 0:
                        nc.scalar.copy(out=ktall[:, t, :], in_=ps)
                    else:
                        nc.vector.tensor_copy(out=ktall[:, t, :], in_=ps)

                wv_sb = wcb.tile([128, FC, HP], FP8, tag="wcb1024", bufs=3, name="wv")
                nc.sync.dma_start(out=wv_sb,
                                  in_=w_v.rearrange("(f p) c -> p f c", p=128))
                vall = acts.tile([128, 2, HP], FP8, tag="vall", name="vall")
                for rt in range(2):
                    ps = ps2.tile([128, HP], F32, tag="ps2", name="ps_v")
                    for cs in (0, 512):
                        for fc in range(0, FC, 2):
                            nc.tensor.matmul(
                                out=ps[:, cs:cs + 512],
                                lhsT=bT6[:, fc:fc + 2, 128 * rt:128 * (rt + 1)],
                                rhs=wv_sb[:, fc:fc + 2, cs:cs + 512],
                                start=(fc == 0), stop=False,
                                perf_mode=PM.DoubleRow)
                        nc.tensor.matmul(out=ps[:, cs:cs + 512],
                                         lhsT=ones1[:, :128],
                                         rhs=vones[:, cs:cs + 512],
                                         start=False, stop=True)
                    nc.scalar.copy(out=vall[:, rt, :], in_=ps)

                # ---------------- AllGather k^T/v within batch group ----------------
                kv_stage = dram.tile([SHARD], BF16)
                kv_gath = dram.tile([4 * SHARD], BF16)
                kst_k = kv_stage[0:KT_SLOTS].bitcast(FP8).rearrange(
                    "(t p c) -> p t c", p=128, c=256)
                kst_v = kv_stage[KT_SLOTS:SHARD].bitcast(FP8).rearrange(
                    "(j p c) -> p j c", p=128, c=HP)
                nc.gpsimd.dma_start(out=kst_k, in_=ktall)
                nc.gpsimd.dma_start(out=kst_v, in_=vall)
                if os.environ.get("KSUB") != "noag":
                    nc.gpsimd.collective_compute(
                        "AllGather", OP.bypass,
                        replica_groups=[[0, 1, 2, 3], [4, 5, 6, 7]],
                        ins=[kv_stage.opt()],
                        outs=[kv_gath.opt()],
                    )
                kvg = kv_gath.rearrange("(r n) -> r n", n=SHARD)
                kt_view = kvg[:, 0:KT_SLOTS].bitcast(FP8).rearrange(
                    "r (t p c) -> t p r c", p=128, c=256)

                ktf = {}

                def load_ktf(t):
                    kf = acts.tile([128, 4, 256], FP8, tag="ktf", bufs=3,
                                   name=f"ktf{t}")
                    nc.sync.dma_start(out=kf, in_=kt_view[t])
                    ktf[t] = kf.rearrange("p r c -> p (r c)")
                vfall = acts.tile([128, 4, 2, HP], FP8, tag="vfall", name="vfall")
                v_gath = kvg[:, KT_SLOTS:SHARD].bitcast(FP8).rearrange(
                    "r (j p c) -> j p r c", p=128, c=HP)
                for j in range(2):
                    nc.sync.dma_start(out=vfall[:, :, j, :], in_=v_gath[j])
                vf = [vfall[:, kt // 2, kt % 2, :] for kt in range(8)]

                if stage == 2:
                    load_ktf(0)
                    srcs = [ktf[0][:, 0:D], vf[0][:, 0:D]]
                    for rt in range(2):
                        yt = tmp.tile([128, D], F32, tag="yt", bufs=2)
                        nc.vector.tensor_copy(out=yt, in_=srcs[rt])
                        nc.sync.dma_start(out=y_out.ap()[128 * rt:128 * (rt + 1), :], in_=yt)
                if stage >= 3:
                    # sigmoid output gates (overlap the collective; only need sT)
                    ps_og = proj_nat(sT6, w_op1, FC, D, bias_row=True)
                    og_sb = [acts.tile([128, D], BF16, tag=f"og{rt}", name=f"og{rt}") for rt in range(2)]
                    for rt in range(2):
                        nc.scalar.activation(out=og_sb[rt], in_=ps_og[rt], func=AF.Sigmoid, scale=IS)
                    ps_opg0 = proj_nat(sT6, w_op2, FC, D, bias_row=True)
                    opg_sb = []
                    for rt in range(2):
                        opg = acts.tile([128, D], BF16, tag=f"opg{rt}", name=f"opg{rt}")
                        nc.scalar.activation(out=opg, in_=ps_opg0[rt], func=AF.Sigmoid, scale=IS)
                        opg_sb.append(opg)
                    # ---------------- q^T, gate^T (overlaps the collective) ----------
                    wq_sb = wcb.tile([128, FC, HP], FP8, tag="wcb1024", bufs=3, name="wq")
                    nc.sync.dma_start(out=wq_sb,
                                      in_=w_q.rearrange("(f p) c -> p f c", p=128))
                    qt_sb = []
                    for t in range(8):
                        ps = ps1.tile([128, 256], F32, tag="ps1", name="ps_cb")
                        for fc in range(0, FC, 2):
                            nc.tensor.matmul(out=ps,
                                             lhsT=wq_sb[:, fc:fc + 2, 128 * t:128 * (t + 1)],
                                             rhs=bT6[:, fc:fc + 2, :],
                                             start=(fc == 0), stop=(fc == FC - 2),
                                             perf_mode=PM.DoubleRow)
                        qt = acts.tile([128, 256], BF16, tag=f"qt{t}")
                        nc.vector.tensor_scalar(out=qt, in0=ps, scalar1=qb[:, t:t + 1],
                                                scalar2=None, op0=OP.add)
                        qt_sb.append(qt)

                    wg_sb = wcb.tile([128, FC, HP], FP8, tag="wcb1024", bufs=3, name="wg")
                    nc.sync.dma_start(out=wg_sb,
                                      in_=w_g.rearrange("(f p) c -> p f c", p=128))
                    gate_g = []
                    for t in range(8):
                        ps = ps1.tile([128, 256], F32, tag="ps1", name="ps_cb")
                        for fc in range(0, FC, 2):
                            nc.tensor.matmul(out=ps,
                                             lhsT=wg_sb[:, fc:fc + 2, 128 * t:128 * (t + 1)],
                                             rhs=bT6[:, fc:fc + 2, :],
                                             start=(fc == 0), stop=(fc == FC - 2),
                                             perf_mode=PM.DoubleRow)
                        gt = acts.tile([128, 256], BF16, tag=f"gt{t}")
                        nc.scalar.activation(out=gt, in_=ps, func=AF.Sigmoid, scale=IS)
                        gate_g.append(gt)


                    # AdaLN2 sn-side projections depend only on snT: run before attention
                    ps_ss2 = proj_nat(snT6, w_ss2, FC, D, bias_row=True)
                    sig2 = [acts.tile([128, D], BF16, tag=f"sig_{rt}", bufs=1, name=f"sig2_{rt}") for rt in range(2)]
                    for rt in range(2):
                        nc.scalar.activation(out=sig2[rt], in_=ps_ss2[rt], func=AF.Sigmoid, scale=IS)
                    ps_sb2 = proj_nat(snT6, w_sb2, FC, D)
                    sb2_sb = [acts.tile([128, D], BF16, tag=f"sb2_{rt}", name=f"sb2_{rt}") for rt in range(2)]
                    for rt in range(2):
                        nc.vector.tensor_scalar_mul(sb2_sb[rt], ps_sb2[rt], IS)

                    # ---------------- attention (grouped normalization, pipelined x) --
                    # the ones-column is host-scaled by S8/256 so the p-sum rows
                    # stay inside fp8 range when xT8 is written with the 1/256 fold
                    xT8 = acts.tile([128, 8, 256], FP8, tag="xT8", name="xT8")
                    for t2 in range(3):  # prefetch gathered k^T for first pairs
                        load_ktf(t2)
                    for t in range(8):  # head pairs
                        if t + 3 <= 7:
                            load_ktf(t + 3)
                        ps_pv = ps1.tile([128, 256], F32, tag="ps1", name="ps_pv")
                        ez_t2 = ezp.tile([128, 2, 8, 256], FP8, tag="ez")
                        nc.sync.dma_start(
                            out=ez_t2,
                            in_=ez_in[2 * t:2 * t + 2].rearrange("h p e r -> p h e r"))
                        for hb in range(2):
                            h = 2 * t + hb
                            base = 64 * hb
                            p_half = []
                            for half in range(2):
                                ez_t = ez_t2[:, hb, 4 * half:4 * half + 4, :]
                                ps_s = ps2.tile([128, 1024], F32, tag="ps2", name="ps_s")
                                for k4 in range(4):
                                    kt = 4 * half + k4
                                    nc.tensor.matmul(
                                        out=ps_s[:, 256 * k4:256 * (k4 + 1)],
                                        lhsT=ktf[t][base:base + 48, 128 * kt:128 * (kt + 1)],
                                        rhs=qt_sb[t][base:base + 48, :],
                                        start=True, stop=True)
                                p = pp.tile([128, 4, 256], FP8, tag="p")
                                nc.scalar.activation(
                                    out=p.rearrange("p a b -> p (a b)"), in_=ps_s,
                                    func=AF.Exp, scale=SCALE / (S8 * S8))
                                eng = nc.vector if (hb + half) % 2 == 0 else nc.gpsimd
                                eng.tensor_mul(p.rearrange("p a b -> p (a b)"),
                                               p.rearrange("p a b -> p (a b)"),
                                               ez_t.rearrange("p a b -> p (a b)"))
                                p_half.append(p)
                            for kt in range(8):
                                nc.tensor.matmul(
                                    out=ps_pv[base:base + 64, :],
                                    lhsT=vf[kt][:, HDP * h:HDP * (h + 1)],
                                    rhs=p_half[kt // 4][:, kt % 4, :],
                                    start=(h % 2 == 0 and kt == 0),
                                    stop=(h % 2 == 1 and kt == 7),
                                    tile_position=(0, base) if hb else None)
                        # row sums sit at partitions 0 / 64 (ones column of padded V);
                        # reciprocal them in place and broadcast via a ones-matmul:
                        # no DMA round-trip, normalize completes per pair.
                        rr = tmp.tile([128, 256], BF16, tag="tsum")
                        with nc.allow_low_precision(reason="per-query 1/sum scale; bf16 matches prior R4b"):
                            nc.vector.reciprocal(out=rr[0:1, :], in_=ps_pv[0:1, :])
                            nc.vector.reciprocal(out=rr[64:65, :], in_=ps_pv[64:65, :])
                        nc.vector.scalar_tensor_tensor(
                            out=xT8[:, t, :], in0=ps_pv, scalar=1.0 / 256.0,
                            in1=gate_g[t], op0=OP.mult, op1=OP.mult)
                        ps_bc = ps1.tile([128, 256], F32, tag="ps1", name="ps_bc")
                        nc.tensor.matmul(out=ps_bc[0:64, :], lhsT=onesf[0:1, :],
                                         rhs=rr[0:1, :], start=True, stop=True)
                        nc.tensor.matmul(out=ps_bc[64:128, :], lhsT=onesf[64:65, :],
                                         rhs=rr[64:65, :], start=True, stop=True,
                                         tile_position=(64, 64))
                        nc.vector.tensor_mul(xT8[:, t, :], xT8[:, t, :], ps_bc)

                    # output projection: x = xT.T @ o_w (starts as soon as slots free)
                    ps_x = [ps2.tile([128, D], F32, tag="ps2", name="ps_x") for _ in range(2)]
                    wt_ow = wp.tile([128, 8, D], FP8, tag="wow8", bufs=1, name="wt_ow")
                    nc.sync.dma_start(
                        out=wt_ow, in_=w_ow.rearrange("(f p) c -> p f c", p=128))
                    for tq in range(0, 8, 2):
                        for rt in range(2):
                            for cs in (0, 512):
                                ce = min(cs + 512, D)
                                nc.tensor.matmul(
                                    out=ps_x[rt][:, cs:ce],
                                    lhsT=xT8[:, tq:tq + 2, 128 * rt:128 * (rt + 1)],
                                    rhs=wt_ow[:, tq:tq + 2, cs:ce],
                                    start=(tq == 0), stop=(tq == 6),
                                    perf_mode=PM.DoubleRow)

                    a1_sb = []
                    for rt in range(2):
                        eng_r = nc.vector if rt == 0 else nc.gpsimd
                        xg = tmp.tile([128, D], BF16, tag="xg")
                        nc.vector.scalar_tensor_tensor(out=xg, in0=ps_x[rt], scalar=IS, in1=og_sb[rt], op0=OP.mult, op1=OP.mult)
                        a1 = acts.tile([128, D], F32, tag=f"a1_{rt}")
                        eng_r.tensor_add(a1, a_sb[rt], xg)
                        a1_sb.append(a1)

                    if stage == 3:
                        for rt in range(2):
                            nc.sync.dma_start(out=y_out.ap()[128 * rt:128 * (rt + 1), :], in_=a1_sb[rt])
                    if stage >= 4:
                        # ---------------- AdaLN 2 (sn reused: snw folded on host) --------
                        an2 = [acts.tile([128, D], BF16, tag="lnout", bufs=4, name=f"an2_{rt}") for rt in range(2)]
                        layernorm(an2, a1_sb)
                        a2_sb = [acts.tile([128, D], BF16, tag=f"ba2_{rt}", bufs=1, name=f"a2_{rt}") for rt in range(2)]
                        for rt in range(2):
                            eng_r = nc.vector if rt == 0 else nc.gpsimd
                            tt = tmp.tile([128, D], BF16, tag="ttmp")
                            eng_r.tensor_mul(tt, an2[rt], sig2[rt])
                            eng_r.tensor_add(a2_sb[rt], tt, sb2_sb[rt])
                        a2T6 = acts.tile([128, FC, 256], FP8, tag="a2T6", name="a2T6")
                        transpose6(a2T6, a2_sb)

                        # ---------------- transition (feature-on-partition) --------------
                        def proj_convB(w_dram, rhs_tiles, n_oct, wtagbase):
                            wt3 = wcb.tile([128, FC, HID], FP8, tag="wcbBIG", bufs=2,
                                           name=wtagbase)
                            nc.sync.dma_start(
                                out=wt3,
                                in_=w_dram.rearrange("(f p) c -> p f c", p=128))
                            outs = []
                            for t in range(n_oct):
                                ps = ps1.tile([128, 256], F32, tag="ps1", name="ps_cb")
                                for fc in range(0, FC, 2):
                                    nc.tensor.matmul(out=ps,
                                                     lhsT=wt3[:, fc:fc + 2, 128 * t:128 * (t + 1)],
                                                     rhs=rhs_tiles[:, fc:fc + 2, :],
                                                     start=(fc == 0), stop=(fc == FC - 2),
                                                     perf_mode=PM.DoubleRow)
                                outs.append(ps)
                            return outs

                        # (op gate hoisted to kernel start)
                        hT12 = acts.tile([128, 12, 256], FP8, tag="hT12", name="hT12")
                        u_sb = []
                        for t, ps in enumerate(proj_convB(w_swu, a2T6, 12, "wsu")):
                            ut = acts.tile([128, 256], BF16, tag=f"u{t}", name=f"u{t}")
                            if t % 2 == 0:
                                nc.vector.tensor_copy(out=ut, in_=ps)
                            else:
                                nc.scalar.copy(out=ut, in_=ps)
                            u_sb.append(ut)
                        sg_sb = []
                        for t, ps in enumerate(proj_convB(w_swg, a2T6, 12, "wsg")):
                            st_ = acts.tile([128, 256], BF16, tag=f"sg{t}", name=f"sg{t}")
                            nc.scalar.activation(out=st_, in_=ps, func=AF.Silu, scale=IS)
                            sg_sb.append(st_)
                        ps_t = [ps2.tile([128, D], F32, tag="ps2", name="ps_t") for _ in range(2)]
                        wt_ba = wp.tile([128, 12, D], FP8, tag="wba12", bufs=1,
                                        name="wt_ba")
                        nc.sync.dma_start(
                            out=wt_ba, in_=w_ba.rearrange("(f p) c -> p f c", p=128))
                        for t, ps in enumerate(proj_convB(w_ab, a2T6, 12, "wab")):
                            eng_h = nc.vector if t % 2 == 0 else nc.gpsimd
                            hu = tmp.tile([128, 256], BF16, tag="hu")
                            eng_h.tensor_mul(hu, sg_sb[t], u_sb[t])
                            nc.vector.scalar_tensor_tensor(
                                out=hT12[:, t, :], in0=ps, scalar=IS,
                                in1=hu, op0=OP.mult, op1=OP.mult)
                            if t % 2 == 1:
                                for rt in range(2):
                                    for cs in (0, 512):
                                        ce = min(cs + 512, D)
                                        nc.tensor.matmul(
                                            out=ps_t[rt][:, cs:ce],
                                            lhsT=hT12[:, t - 1:t + 1, 128 * rt:128 * (rt + 1)],
                                            rhs=wt_ba[:, t - 1:t + 1, cs:ce],
                                            start=(t == 1), stop=(t == 11),
                                            perf_mode=PM.DoubleRow)

                        for rt in range(2):
                            eng_r = nc.vector if rt == 0 else nc.gpsimd
                            yt = tmp.tile([128, D], F32, tag="yt", bufs=2)
                            for (hs, he) in ((0, 384), (384, D)):
                                tg = tmp.tile([128, 384], BF16, tag="tg")
                                nc.vector.scalar_tensor_tensor(
                                    out=tg, in0=ps_t[rt][:, hs:he], scalar=IS2,
                                    in1=opg_sb[rt][:, hs:he], op0=OP.mult, op1=OP.mult)
                                eng_r.tensor_add(
                                    yt[:, hs:he], a1_sb[rt][:, hs:he], tg)
                                nc.sync.dma_start(
                                    out=y_out.ap()[128 * rt:128 * (rt + 1), hs:he],
                                    in_=yt[:, hs:he])

    nc.finalize()
    return nc


def _get_nc():
    if "nc" not in _NC_CACHE:
        _NC_CACHE["nc"] = _build_nc()
    return _NC_CACHE["nc"]


def _pad_cols(w):
    """[768, 768] -> [768, 1024]: each head's 48 cols at a 64-aligned block."""
    wp = np.zeros((D, HP), np.float32)
    wp.reshape(D, H, HDP)[:, :, :HD] = np.asarray(w, np.float32).reshape(D, H, HD)
    return wp


def _bf(x):
    return np.ascontiguousarray(np.asarray(x, np.float32).astype(NPBF16))


def _f8(x):
    """fp8 weights: pre-scaled by S8 (consumers divide back out)."""
    return np.ascontiguousarray(
        (np.asarray(x, np.float32) * S8).astype(NPF8))


def _bytes(x):
    return np.ascontiguousarray(x).view(np.uint8).ravel()


def kernel(**inputs):
    a = np.asarray(inputs["a"], np.float32)
    s = np.asarray(inputs["s"], np.float32)
    z = np.asarray(inputs["z"], np.float32)

    snw1 = np.asarray(inputs["adaln1_snw"], np.float32)[:, None]
    snw2 = np.asarray(inputs["adaln2_snw"], np.float32)[:, None]
    w_ss1 = _f8(np.vstack([snw1 * np.asarray(inputs["adaln1_ssw"], np.float32),
                           np.asarray(inputs["adaln1_ssb"], np.float32)[None]]))
    w_sb1 = _f8(snw1 * np.asarray(inputs["adaln1_sbw"], np.float32))
    w_ss2 = _f8(np.vstack([snw2 * np.asarray(inputs["adaln2_ssw"], np.float32),
                           np.asarray(inputs["adaln2_ssb"], np.float32)[None]]))
    w_sb2 = _f8(snw2 * np.asarray(inputs["adaln2_sbw"], np.float32))

    w_q = _f8(_pad_cols(inputs["q_w"]))   # SCALE folded into the exp() scale
    qb_p = np.zeros((H, HDP), np.float32)
    qb_p[:, :HD] = np.asarray(inputs["q_b"], np.float32).reshape(H, HD) * S8
    qb_p = np.ascontiguousarray(qb_p.reshape(8, 128))
    w_k = _f8(_pad_cols(inputs["k_w"]))
    w_g = _f8(_pad_cols(inputs["g_w"]))
    w_vp = np.zeros((D, HP), np.float32)
    w_vp.reshape(D, H, HDP)[:, :, 1:HD + 1] = \
        np.asarray(inputs["v_w"], np.float32).reshape(D, H, HD)
    w_v = _f8(w_vp)
    w_ow = np.zeros((HP, D), np.float32)
    w_ow.reshape(H, HDP, D)[:, 1:HD + 1, :] = \
        np.asarray(inputs["o_w"], np.float32).reshape(H, HD, D)
    w_ow = _f8(w_ow)
    w_op1 = _f8(np.vstack([np.asarray(inputs["outproj_w"], np.float32),
                           np.asarray(inputs["outproj_b"], np.float32)[None]]))
    w_op2 = _f8(np.vstack([np.asarray(inputs["op_w"], np.float32),
                           np.asarray(inputs["op_b"], np.float32)[None]]))
    sw = np.asarray(inputs["swish_w"], np.float32)
    w_swu = _f8(sw[:, :HID])
    w_swg = _f8(sw[:, HID:])
    w_ab = _f8(inputs["a2b_w"])
    w_ba = _f8(inputs["b2a_w"])

    ident = _bf(np.eye(128))
    sel = np.zeros((4, 4, 2, 128), np.float32)
    for g in range(4):
        for r in range(4):
            for p in range(2):
                for m in range(128):
                    if r == 2 * p + m // 64:
                        sel[g, r, p, m] = 1.0
    sel = _bf(sel.reshape(4, 4, 256).transpose(1, 0, 2))
    vones = np.zeros((1, HP), np.float32)
    vones.reshape(H, HDP)[:, 0] = S8 / 256.0  # keeps p-sum rows fp8-safe
    vones = _bf(vones)

    shared = dict(
        ident=ident, sel=sel, vones=vones, qb=qb_p,
        w_ss1=w_ss1, w_sb1=w_sb1, w_ss2=w_ss2, w_sb2=w_sb2,
        w_q=w_q, w_k=w_k, w_g=w_g, w_v=w_v, w_ow=w_ow,
        w_op1=w_op1, w_op2=w_op2, w_swu=w_swu, w_swg=w_swg,
        w_ab=w_ab, w_ba=w_ba,
    )
    # shared tail of the blob (everything after a/qb/s/ez): built once
    tail = np.concatenate([_bytes(shared[name]) for name, _, _ in _LAYOUT[4:]])

    in_maps = []
    for c in range(8):
        beta, q0 = c // 4, 256 * (c % 4)
        rows = slice(q0, q0 + 256)
        ez = np.exp(z[:, beta, rows, :])          # [16, 256, 1024]
        ez = ez.transpose(0, 2, 1)                # [16, 1024k, 256q]
        ez = ez.reshape(H, 8, 128, R).transpose(0, 2, 1, 3)  # [16,128,8,256]
        head = np.concatenate([
            _bytes(np.ascontiguousarray(a[beta, rows, :], np.float32)),
            _bytes(qb_p),
            _bytes(_bf(s[beta, rows, :])),
            _bytes(ez.astype(NPF8)),
        ])
        full = np.concatenate([head, tail])
        assert full.size == 2 * _BLOB_LEN, (full.size, 2 * _BLOB_LEN)
        in_maps.append({"blob": full.view(NPBF16)})

    nc = _get_nc()
    global _LAST_IN_MAPS
    _LAST_IN_MAPS = in_maps
    res = run_bass_kernel_spmd(nc, in_maps, core_ids=list(range(8)))

    out = np.empty((B, N, D), np.float32)
    for c in range(8):
        beta, q0 = c // 4, 256 * (c % 4)
        out[beta, q0:q0 + 256, :] = res.results[c]["y"]
    return out

